# revision 16
# baseline (speedup 1.0000x reference)
"""Trainium2 Bass kernel for nn_AMCValueNet (ragged prefix-attention value net).

Math: the reference's [n-1, n, n] masked-softmax prefix attention collapses to
dense matmuls.  With S = (q @ k.T)/sqrt(d) and E = exp(S) (scores are O(1), no
max-subtraction needed):

  Lc[i,j] = sum_{k<j} E[i,k]            = E @ U        (U strict upper ones)
  Bt[i,j] = 1{i<j} / Lc[i,j]
  C[j,k]  = sum_{i<j} E[i,k]/Lc[i,j]    -> Ct = E.T @ Bt  ([k, j] layout)
  out     = sum_j (1/j) sum_{k<j} Ct[k,j] * z[k]  +  w2 . sum_i x_i + n*bc
  where z = v @ w1 = x @ (Wv.T @ w1) + bv.w1,  Wc = [w1 | w2].

Sharding: rows i (queries) are split into 8 contiguous bands of 40; each core
computes the full K projection (needed by every band) but only its band of Q,
S, E, Lc, Bt and a partial Ct / partial output scalar.  Host sums 8 scalars.
"""

import os
import numpy as np
import ml_dtypes

import concourse.bass as bass
import concourse.bacc as bacc
import concourse.mybir as mybir
from concourse import tile
from concourse.bass_utils import run_bass_kernel_spmd

N = 320
D = 512
NCORES = 8
B = N // NCORES          # 40 query rows per core
PT = 128                 # partition tile
ND = D // PT             # 4 d-chunks
KT_SIZES = [128, 128, 64]  # k tiles covering 320
SCALE = 1.0 / float(np.sqrt(np.float32(D)))

F32 = mybir.dt.float32
BF16 = mybir.dt.bfloat16
BF16_NP = ml_dtypes.bfloat16

LAST_RESULT = None  # BassKernelResults of the most recent run (for test.py)
_CACHED_NC = None


def _ensure_ntff_hook():
    """Install the antenv.axon_hooks NTFF-profile shim if the container's
    antenv stub lacks it (mirrors trn_boot._ntff_profile_via_ctypes)."""
    import contextlib
    import ctypes
    import sys
    import types

    try:
        from antenv.axon_hooks import get_axon_ntff_profile_hook  # noqa: F401
        return
    except ImportError:
        pass
    so_path = "/opt/axon/libaxon_pjrt.so"
    if not os.path.exists(so_path):
        return
    lib = ctypes.CDLL(so_path)
    if not hasattr(lib, "axon_start_nrt_profile"):
        return
    lib.axon_start_nrt_profile.argtypes = [
        ctypes.POINTER(ctypes.c_int64), ctypes.c_size_t]
    lib.axon_start_nrt_profile.restype = ctypes.c_int64
    lib.axon_stop_nrt_profile.argtypes = [ctypes.c_char_p]
    lib.axon_stop_nrt_profile.restype = ctypes.c_int64

    @contextlib.contextmanager
    def _hook(output_dir, device_ids):
        import jax
        jax.devices()
        if device_ids:
            ids = (ctypes.c_int64 * len(device_ids))(*device_ids)
            rc = lib.axon_start_nrt_profile(ids, len(device_ids))
        else:
            rc = lib.axon_start_nrt_profile(None, 0)
        if rc != 0:
            raise RuntimeError(f"axon_start_nrt_profile rc={rc}")
        try:
            yield
        finally:
            n = lib.axon_stop_nrt_profile(str(output_dir).encode())
            print(f"profile: {n} ntff file(s) -> {output_dir}", file=sys.stderr)

    mod = types.ModuleType("antenv.axon_hooks")
    mod.get_axon_ntff_profile_hook = lambda: _hook
    mod.set_axon_ntff_profile_hook = lambda h: None
    import antenv
    antenv.axon_hooks = mod
    sys.modules["antenv.axon_hooks"] = mod


def _build_nc():
    stage = int(os.environ.get("KERNEL_STAGE", "9"))
    nc = bacc.Bacc("TRN2", target_bir_lowering=False, debug=False)

    xT = nc.dram_tensor("xT", [D, N], BF16, kind="ExternalInput")
    xTb = nc.dram_tensor("xTb", [D, B], BF16, kind="ExternalInput")
    wqT = nc.dram_tensor("wqT", [D, D], BF16, kind="ExternalInput")
    wkT = nc.dram_tensor("wkT", [D, D], BF16, kind="ExternalInput")
    bqc = nc.dram_tensor("bqc", [PT, ND], F32, kind="ExternalInput")
    bkc = nc.dram_tensor("bkc", [PT, ND], F32, kind="ExternalInput")
    zc = nc.dram_tensor("zc", [PT, 3], F32, kind="ExternalInput")
    rj = nc.dram_tensor("rj", [1, N], F32, kind="ExternalInput")
    xsc = nc.dram_tensor("xsc", [1, D], F32, kind="ExternalInput")
    w2c = nc.dram_tensor("w2c", [1, D], F32, kind="ExternalInput")
    mbig = nc.dram_tensor("mbig", [B, N], F32, kind="ExternalInput")
    out_d = nc.dram_tensor("out", [1, 1], F32, kind="ExternalOutput")
    dbg = int(os.environ.get("KERNEL_DBG", "0"))
    dbg_d = None
    if dbg:
        dbg_d = nc.dram_tensor("dbg", [PT, 3, N], F32, kind="ExternalOutput")

    with tile.TileContext(nc) as tc:
        with (
            tc.tile_pool(name="w", bufs=1) as wpool,
            tc.tile_pool(name="act", bufs=1) as apool,
            tc.tile_pool(name="small", bufs=1) as spool,
            tc.tile_pool(name="g", bufs=3) as gpool,
            tc.tile_pool(name="pa", bufs=2, space="PSUM") as pa,   # [128,320]
            tc.tile_pool(name="pb", bufs=2, space="PSUM") as pb,   # [128,40]
            tc.tile_pool(name="pc", bufs=2, space="PSUM") as pc,   # [40,320]
            tc.tile_pool(name="pd", bufs=1, space="PSUM") as pd,   # [1,320]
        ):
            last = None  # f32-able AP for debug-stage output

            # ---- constant / input tiles ----
            xT_sb = wpool.tile([PT, ND, N], BF16, tag="xT")
            xTb_sb = wpool.tile([PT, ND, B], BF16, tag="xTb")
            wq_sb = wpool.tile([PT, ND, D], BF16, tag="wq")
            wk_sb = wpool.tile([PT, ND, D], BF16, tag="wk")
            for t in range(ND):
                nc.sync.dma_start(xTb_sb[:, t, :], xTb[t * PT:(t + 1) * PT, :])
                nc.sync.dma_start(wq_sb[:, t, :], wqT[t * PT:(t + 1) * PT, :])
                nc.sync.dma_start(xT_sb[:, t, :], xT[t * PT:(t + 1) * PT, :])
                nc.sync.dma_start(wk_sb[:, t, :], wkT[t * PT:(t + 1) * PT, :])
            bq_sb = spool.tile([PT, ND], F32, tag="bq")
            bk_sb = spool.tile([PT, ND], F32, tag="bk")
            zc_sb = spool.tile([PT, 3], F32, tag="zc")
            rj_sb = spool.tile([1, N], F32, tag="rj")
            xs_sb = spool.tile([1, D], F32, tag="xs")
            w2_sb = spool.tile([1, D], F32, tag="w2")
            mb_sb = spool.tile([B, N], F32, tag="mb")
            nc.sync.dma_start(bq_sb[:], bqc[:, :])
            nc.sync.dma_start(bk_sb[:], bkc[:, :])
            nc.sync.dma_start(zc_sb[:], zc[:, :])
            nc.sync.dma_start(rj_sb[:], rj[:, :])
            nc.sync.dma_start(xs_sb[:], xsc[:, :])
            nc.sync.dma_start(w2_sb[:], w2c[:, :])
            nc.sync.dma_start(mb_sb[:], mbig[:, :])
            last = mb_sb[0:1, 0:1]

            # U[k, j] = 1{k < j} (strict upper), built on-chip per k-tile
            u_sb = spool.tile([PT, 3, N], BF16, tag="u")
            ones_bf = spool.tile([PT, 1], BF16, tag="ones_bf")
            nc.gpsimd.memset(ones_bf[:], 1.0)
            if stage >= 1:
                for kt, kn in enumerate(KT_SIZES):
                    nc.gpsimd.memset(u_sb[:kn, kt, :], 1.0)
                    # keep 1.0 where j - k_global > 0 (i.e. k < j), else 0
                    nc.gpsimd.affine_select(
                        out=u_sb[:kn, kt, :], in_=u_sb[:kn, kt, :],
                        compare_op=mybir.AluOpType.is_gt, fill=0.0,
                        base=-(kt * PT), channel_multiplier=-1,
                        pattern=[[1, N]],
                    )
                last = u_sb[0:1, 0, 0:1]

            if stage >= 2:
                # ---- K/Q projections ----
                kproj_sb = apool.tile([PT, ND, N], BF16, tag="kproj")
                qproj_sb = apool.tile([PT, ND, B], BF16, tag="qproj")
                for q in range(ND):
                    ps = pa.tile([PT, N], F32, tag="pa")
                    for dk in range(ND):
                        nc.tensor.matmul(
                            ps[:], wk_sb[:, dk, q * PT:(q + 1) * PT],
                            xT_sb[:, dk, :],
                            start=(dk == 0), stop=(dk == ND - 1),
                        )
                    nc.scalar.activation(
                        kproj_sb[:, q, :], ps[:],
                        mybir.ActivationFunctionType.Identity,
                        bias=bk_sb[:, q:q + 1],
                    )
                for q in range(ND):
                    ps = pb.tile([PT, B], F32, tag="pb")
                    for dk in range(ND):
                        nc.tensor.matmul(
                            ps[:], wq_sb[:, dk, q * PT:(q + 1) * PT],
                            xTb_sb[:, dk, :],
                            start=(dk == 0), stop=(dk == ND - 1),
                        )
                    nc.vector.tensor_scalar_add(
                        qproj_sb[:, q, :], ps[:], bq_sb[:, q:q + 1])
                last = kproj_sb[0:1, 0, 0:1]

            if stage >= 3:
                # ---- S band and exp ----
                s_ps = pc.tile([B, N], F32, tag="pc")
                for q in range(ND):
                    nc.tensor.matmul(s_ps[:], qproj_sb[:, q, :],
                                     kproj_sb[:, q, :],
                                     start=(q == 0), stop=(q == ND - 1))
                e_sb = apool.tile([B, N], BF16, tag="e")
                nc.scalar.activation(e_sb[:], s_ps[:],
                                     mybir.ActivationFunctionType.Exp,
                                     scale=SCALE)
                last = e_sb[0:1, 0:1]

            if stage >= 4:
                et_sb = apool.tile([PT, 3, B], BF16, tag="et")
                for kt, kn in enumerate(KT_SIZES):
                    ps = pb.tile([PT, B], F32, tag="pb")
                    for q in range(ND):
                        nc.tensor.matmul(
                            ps[:kn, :], kproj_sb[:, q, kt * PT:kt * PT + kn],
                            qproj_sb[:, q, :], start=(q == 0),
                            stop=(q == ND - 1),
                        )
                    nc.scalar.activation(et_sb[:kn, kt, :], ps[:kn, :],
                                         mybir.ActivationFunctionType.Exp,
                                         scale=SCALE)
                last = et_sb[0:1, 0, 0:1]

            if stage >= 5:
                # ---- Lc -> Bt ----
                lc_ps = pc.tile([B, N], F32, tag="pc")
                for kt, kn in enumerate(KT_SIZES):
                    nc.tensor.matmul(lc_ps[:], et_sb[:kn, kt, :],
                                     u_sb[:kn, kt, :],
                                     start=(kt == 0), stop=(kt == 2))
                lcm_sb = apool.tile([B, N], F32, tag="lcm")
                nc.vector.tensor_add(lcm_sb[:], lc_ps[:], mb_sb[:])
                bt_sb = apool.tile([B, N], BF16, tag="bt")
                with nc.allow_low_precision(reason="bf16 Bt validated, 1e-3"):
                    nc.vector.reciprocal(bt_sb[:], lcm_sb[:])
                last = bt_sb[0:1, 0:1]

            if stage >= 6:
                # ---- Ct = E.T @ Bt ; G = mask(Ct) * z ; D = colsum(G) ----
                d_ps = pd.tile([1, N], F32, tag="pd")
                g_tiles = []
                for kt, kn in enumerate(KT_SIZES):
                    ct_ps = pa.tile([PT, N], F32, tag="pa")
                    nc.tensor.matmul(ct_ps[:kn, :],
                                     e_sb[:, kt * PT:kt * PT + kn], bt_sb[:])
                    g_sb = gpool.tile([PT, N], BF16, tag="g")
                    nc.vector.tensor_scalar_mul(g_sb[:kn, :], ct_ps[:kn, :],
                                                zc_sb[:kn, kt:kt + 1])
                    nc.gpsimd.affine_select(
                        out=g_sb[:kn, :], in_=g_sb[:kn, :],
                        compare_op=mybir.AluOpType.is_gt, fill=0.0,
                        base=-(kt * PT), channel_multiplier=-1,
                        pattern=[[1, N]],
                    )
                    g_tiles.append(g_sb)
                    nc.tensor.matmul(d_ps[:], ones_bf[:kn, :], g_sb[:kn, :],
                                     start=(kt == 0), stop=(kt == 2))

            out_sb = spool.tile([1, 1], F32, tag="out")
            if stage >= 7:
                # ---- t1 = sum(D * rj) ; t2 = sum(xs * w2) ----
                junk1 = spool.tile([1, N], F32, tag="junk1")
                t1_sb = spool.tile([1, 1], F32, tag="t1")
                nc.vector.tensor_mul(junk1[:], d_ps[:], rj_sb[:])
                nc.vector.reduce_sum(t1_sb[:], junk1[:],
                                     axis=mybir.AxisListType.X)
                junk2 = spool.tile([1, D], F32, tag="junk2")
                t2_sb = spool.tile([1, 1], F32, tag="t2")
                nc.vector.tensor_mul(junk2[:], xs_sb[:], w2_sb[:])
                nc.vector.reduce_sum(t2_sb[:], junk2[:],
                                     axis=mybir.AxisListType.X)
                nc.vector.tensor_add(out_sb[:], t1_sb[:], t2_sb[:])
            elif stage == 6:
                with nc.allow_low_precision(reason="debug"):
                    nc.vector.tensor_copy(out_sb[:], d_ps[0:1, 0:1])
            else:
                with nc.allow_low_precision(reason="debug"):
                    nc.vector.tensor_copy(out_sb[:], last)
            nc.sync.dma_start(out_d[:, :], out_sb[:])
            if dbg:
                dbg_sb = wpool.tile([PT, 3, N], F32, tag="dbg")
                nc.gpsimd.memset(dbg_sb[:], 0.0)
                with nc.allow_low_precision(reason="debug dump"):
                    if dbg == 1:    # U tiles
                        for kt, kn in enumerate(KT_SIZES):
                            nc.vector.tensor_copy(dbg_sb[:kn, kt, :],
                                                  u_sb[:kn, kt, :])
                    elif dbg == 2:  # E band
                        nc.vector.tensor_copy(dbg_sb[:B, 0, :], e_sb[:])
                    elif dbg == 3:  # Bt band
                        nc.vector.tensor_copy(dbg_sb[:B, 0, :], bt_sb[:])
                    elif dbg == 4:  # G tiles (post-mask)
                        for kt, kn in enumerate(KT_SIZES):
                            nc.vector.tensor_copy(dbg_sb[:kn, kt, :],
                                                  g_tiles[kt][:kn, :])
                    elif dbg == 5:  # ET tiles
                        for kt, kn in enumerate(KT_SIZES):
                            nc.vector.tensor_copy(dbg_sb[:kn, kt, :B],
                                                  et_sb[:kn, kt, :])
                nc.sync.dma_start(dbg_d[:, :, :], dbg_sb[:])

    nc.compile()
    return nc


def _get_nc():
    global _CACHED_NC
    if _CACHED_NC is None:
        _CACHED_NC = _build_nc()
    return _CACHED_NC


def _fold(v, nt):
    """[nt*128] -> [128, nt] column-major fold (v[t*128+p] -> out[p, t])."""
    return np.ascontiguousarray(v.reshape(nt, PT).T.astype(np.float32))


def kernel(**inputs):
    global LAST_RESULT
    x = np.asarray(inputs["x"], np.float32)
    Wq = np.asarray(inputs["Wq"], np.float32)
    bq = np.asarray(inputs["bq"], np.float32)
    Wk = np.asarray(inputs["Wk"], np.float32)
    bk = np.asarray(inputs["bk"], np.float32)
    Wv = np.asarray(inputs["Wv"], np.float32)
    bv = np.asarray(inputs["bv"], np.float32)
    Wc = np.asarray(inputs["Wc"], np.float32)
    bc = np.asarray(inputs["bc"], np.float32)

    w1, w2 = Wc[0, :D], Wc[0, D:]
    xT_bf = np.ascontiguousarray(x.T).astype(BF16_NP)
    wqT_bf = np.ascontiguousarray(Wq.T).astype(BF16_NP)
    wkT_bf = np.ascontiguousarray(Wk.T).astype(BF16_NP)
    z = (x @ (Wv.T @ w1) + bv @ w1).astype(np.float32)
    zpad = np.zeros(3 * PT, np.float32)
    zpad[:N] = z
    rj = np.zeros((1, N), np.float32)
    rj[0, 1:] = 1.0 / np.arange(1, N, dtype=np.float32)

    common = {
        "xT": xT_bf, "wqT": wqT_bf, "wkT": wkT_bf,
        "bqc": _fold(bq, ND), "bkc": _fold(bk, ND),
        "zc": _fold(zpad, 3), "rj": rj,
        "w2c": np.ascontiguousarray(w2[None, :].astype(np.float32)),
    }
    xs_row = np.ascontiguousarray(
        x.sum(axis=0, dtype=np.float64).astype(np.float32)[None, :])
    jidx = np.arange(N)[None, :]
    in_maps = []
    for c in range(NCORES):
        i0 = c * B
        iglob = (i0 + np.arange(B))[:, None]
        mbig = np.where(iglob < jidx, 0.0, 1e30).astype(np.float32)
        m = dict(common)
        m["xTb"] = np.ascontiguousarray(x[i0:i0 + B].T).astype(BF16_NP)
        m["mbig"] = mbig
        # t2 = w2 . sum_i x_i must be counted once: only core 0 gets xs
        m["xsc"] = xs_row if c == 0 else np.zeros((1, D), np.float32)
        in_maps.append(m)

    nc = _get_nc()
    trace = bool(int(os.environ.get("KERNEL_TRACE", "0")))
    trace_cores = None
    if trace:
        try:
            _ensure_ntff_hook()
        except Exception as e:
            print(f"ntff hook shim failed ({e!r}); running untraced")
            trace = False
        if int(os.environ.get("KERNEL_TRACE_ALL", "0")):
            trace_cores = list(range(NCORES))
    res = run_bass_kernel_spmd(
        nc, in_maps, core_ids=list(range(NCORES)),
        trace=trace, trace_cores=trace_cores,
    )
    LAST_RESULT = res
    total = np.float64(0.0)
    for c in range(NCORES):
        total += np.float64(res.results[c]["out"][0, 0])
    total += np.float64(N) * np.float64(bc[0])
    return np.array([total], dtype=np.float32)


# revision 18
# speedup vs baseline: 1.2610x; 1.2610x over previous
"""Trainium2 Bass kernel for nn_AMCValueNet (ragged prefix-attention value net).

Math: the reference's [n-1, n, n] masked-softmax prefix attention collapses to
dense ops.  With S = (q @ k.T)/sqrt(d) and E = exp(S) (scores are O(1), no
max-subtraction needed):

  Lc[i,j]  = sum_{k<j} E[i,k]                (row prefix-scan of E)
  Bt[i,j]  = 1{i<j} / Lc[i,j]
  Ct[k,j]  = sum_i E[i,k] Bt[i,j]            (one [n,n] matmul)
  t1       = sum_{k,j} Ct[k,j] * z[k] * (1/j) * 1{k<j}
  out      = t1 + w2 . sum_i x_i + n*bc
  where z = v @ w1 = x @ (Wv.T @ w1) + bv.w1,  Wc = [w1 | w2].

The z[k]*(1/j)*1{k<j} factor is a host-precomputed bf16 "zmask" so t1 is just
(elementwise mul) + (ones colsum matmul) + (row reduce).

Sharding: query rows i are split into 8 contiguous bands of 40; each core
computes the full K projection (every band needs all keys) plus its band of
Q/S/E/scan/Bt and a partial t1.  The host sums the 8 partial scalars.
"""

import os
import numpy as np
import ml_dtypes

import concourse.bass as bass
import concourse.bacc as bacc
import concourse.mybir as mybir
from concourse import tile
from concourse.bass_utils import run_bass_kernel_spmd

N = 320
D = 512
NCORES = 8
B = N // NCORES          # 40 query rows per core
PT = 128                 # partition tile
ND = D // PT             # 4 d-chunks
KT_SIZES = [128, 128, 64]  # k tiles covering 320
SCALE = 1.0 / float(np.sqrt(np.float32(D)))

F32 = mybir.dt.float32
BF16 = mybir.dt.bfloat16
BF16_NP = ml_dtypes.bfloat16

LAST_RESULT = None  # BassKernelResults of the most recent run (for test.py)
_CACHED_NC = None


def _ensure_ntff_hook():
    """Install the antenv.axon_hooks NTFF-profile shim if the container's
    antenv stub lacks it (mirrors trn_boot._ntff_profile_via_ctypes)."""
    import contextlib
    import ctypes
    import sys
    import types

    try:
        from antenv.axon_hooks import get_axon_ntff_profile_hook  # noqa: F401
        return
    except ImportError:
        pass
    so_path = "/opt/axon/libaxon_pjrt.so"
    if not os.path.exists(so_path):
        return
    lib = ctypes.CDLL(so_path)
    if not hasattr(lib, "axon_start_nrt_profile"):
        return
    lib.axon_start_nrt_profile.argtypes = [
        ctypes.POINTER(ctypes.c_int64), ctypes.c_size_t]
    lib.axon_start_nrt_profile.restype = ctypes.c_int64
    lib.axon_stop_nrt_profile.argtypes = [ctypes.c_char_p]
    lib.axon_stop_nrt_profile.restype = ctypes.c_int64

    @contextlib.contextmanager
    def _hook(output_dir, device_ids):
        import jax
        jax.devices()
        if device_ids:
            ids = (ctypes.c_int64 * len(device_ids))(*device_ids)
            rc = lib.axon_start_nrt_profile(ids, len(device_ids))
        else:
            rc = lib.axon_start_nrt_profile(None, 0)
        if rc != 0:
            raise RuntimeError(f"axon_start_nrt_profile rc={rc}")
        try:
            yield
        finally:
            n = lib.axon_stop_nrt_profile(str(output_dir).encode())
            print(f"profile: {n} ntff file(s) -> {output_dir}", file=sys.stderr)

    mod = types.ModuleType("antenv.axon_hooks")
    mod.get_axon_ntff_profile_hook = lambda: _hook
    mod.set_axon_ntff_profile_hook = lambda h: None
    import antenv
    antenv.axon_hooks = mod
    sys.modules["antenv.axon_hooks"] = mod


def _build_nc():
    nc = bacc.Bacc("TRN2", target_bir_lowering=False, debug=False)

    xT = nc.dram_tensor("xT", [D, N], BF16, kind="ExternalInput")
    xTb = nc.dram_tensor("xTb", [D, B], BF16, kind="ExternalInput")
    wqT = nc.dram_tensor("wqT", [D, D], BF16, kind="ExternalInput")
    wkT = nc.dram_tensor("wkT", [D, D], BF16, kind="ExternalInput")
    smalls = nc.dram_tensor("smalls", [PT, 8], F32, kind="ExternalInput")
    rows = nc.dram_tensor("rows", [1, 2 * D], F32, kind="ExternalInput")
    zmask = nc.dram_tensor("zmask", [PT, 3, N], BF16, kind="ExternalInput")
    bmask = nc.dram_tensor("bmask", [B, N], BF16, kind="ExternalInput")
    out_d = nc.dram_tensor("out", [1, 1], F32, kind="ExternalOutput")

    with tile.TileContext(nc) as tc:
        with (
            tc.tile_pool(name="w", bufs=1) as wpool,
            tc.tile_pool(name="act", bufs=1) as apool,
            tc.tile_pool(name="small", bufs=1) as spool,
            tc.tile_pool(name="g", bufs=3) as gpool,
            tc.tile_pool(name="pa", bufs=2, space="PSUM") as pa,   # [128,320]
            tc.tile_pool(name="pb", bufs=2, space="PSUM") as pb,   # [128,40]
            tc.tile_pool(name="pc", bufs=1, space="PSUM") as pc,   # [40,320]
            tc.tile_pool(name="pd", bufs=1, space="PSUM") as pd,   # [1,320]
        ):
            # ---- input DMAs: one per tensor, spread across sequencers ----
            wk_sb = wpool.tile([PT, ND, D], BF16, tag="wk")
            wq_sb = wpool.tile([PT, ND, D], BF16, tag="wq")
            xT_sb = wpool.tile([PT, ND, N], BF16, tag="xT")
            xTb_sb = wpool.tile([PT, ND, B], BF16, tag="xTb")
            sm_sb = spool.tile([PT, 8], F32, tag="sm")
            rows_sb = spool.tile([1, 2 * D], F32, tag="rows")
            zm_sb = wpool.tile([PT, 3, N], BF16, tag="zm")
            bm_sb = spool.tile([B, N], BF16, tag="bm")

            nc.sync.dma_start(wk_sb[:], wkT.rearrange("(t p) n -> p t n", p=PT))
            nc.sync.dma_start(xT_sb[:], xT.rearrange("(t p) k -> p t k", p=PT))
            nc.scalar.dma_start(wq_sb[:], wqT.rearrange("(t p) n -> p t n", p=PT))
            nc.scalar.dma_start(zm_sb[:], zmask.rearrange("p t n -> p t n"))
            nc.gpsimd.dma_start(xTb_sb[:], xTb.rearrange("(t p) i -> p t i", p=PT))
            nc.gpsimd.dma_start(sm_sb[:], smalls[:, :])
            nc.gpsimd.dma_start(bm_sb[:], bmask[:, :])
            nc.gpsimd.dma_start(rows_sb[:], rows[:, :])

            ones_bf = spool.tile([PT, 1], BF16, tag="ones_bf")
            nc.gpsimd.memset(ones_bf[:], 1.0)

            # ---- projections: PROJ[n, i] = sum_d W.T[d, n] x.T[d, i] + b ----
            kproj_sb = apool.tile([PT, ND, N], BF16, tag="kproj")
            qproj_sb = apool.tile([PT, ND, B], BF16, tag="qproj")
            for q in range(ND):
                ps = pb.tile([PT, B], F32, tag="pb")
                for dk in range(ND):
                    nc.tensor.matmul(
                        ps[:], wq_sb[:, dk, q * PT:(q + 1) * PT],
                        xTb_sb[:, dk, :],
                        start=(dk == 0), stop=(dk == ND - 1),
                    )
                nc.vector.tensor_scalar_add(
                    qproj_sb[:, q, :], ps[:], sm_sb[:, q:q + 1])
            for q in range(ND):
                ps = pa.tile([PT, N], F32, tag="pa")
                for dk in range(ND):
                    nc.tensor.matmul(
                        ps[:], wk_sb[:, dk, q * PT:(q + 1) * PT],
                        xT_sb[:, dk, :],
                        start=(dk == 0), stop=(dk == ND - 1),
                    )
                nc.scalar.activation(
                    kproj_sb[:, q, :], ps[:],
                    mybir.ActivationFunctionType.Identity,
                    bias=sm_sb[:, 4 + q:5 + q],
                )

            # ---- S band [B, N]; E = exp(S/sqrt(d)) in bf16 ----
            s_ps = pc.tile([B, N], F32, tag="pc")
            for q in range(ND):
                nc.tensor.matmul(s_ps[:], qproj_sb[:, q, :], kproj_sb[:, q, :],
                                 start=(q == 0), stop=(q == ND - 1))
            e_sb = apool.tile([B, N], BF16, tag="e")
            nc.scalar.activation(e_sb[:], s_ps[:],
                                 mybir.ActivationFunctionType.Exp, scale=SCALE)

            # ---- prefix-scan -> reciprocal -> masked Bt (bf16) ----
            linc_sb = apool.tile([B, N], F32, tag="linc")
            nc.vector.tensor_tensor_scan(
                out=linc_sb[:, 0:N - 1], data0=e_sb[:, 0:N - 1],
                data1=e_sb[:, 0:N - 1], initial=0.0,
                op0=mybir.AluOpType.add, op1=mybir.AluOpType.bypass,
            )
            rec_sb = apool.tile([B, N], F32, tag="rec")
            nc.vector.reciprocal_approx_fast(
                out=rec_sb[:, 0:N - 1], in_=linc_sb[:, 0:N - 1])
            bt_sb = apool.tile([B, N], BF16, tag="bt")
            nc.gpsimd.memset(bt_sb[:, 0:1], 0.0)
            nc.vector.tensor_mul(bt_sb[:, 1:N], rec_sb[:, 0:N - 1],
                                 bm_sb[:, 1:N])

            # ---- Ct = E.T @ Bt ; G = Ct * zmask ; t1 = sum(G) ----
            d_ps = pd.tile([1, N], F32, tag="pd")
            for kt, kn in enumerate(KT_SIZES):
                ct_ps = pa.tile([PT, N], F32, tag="pa")
                nc.tensor.matmul(ct_ps[:kn, :],
                                 e_sb[:, kt * PT:kt * PT + kn], bt_sb[:])
                g_sb = gpool.tile([PT, N], BF16, tag="g")
                nc.vector.tensor_mul(g_sb[:kn, :], ct_ps[:kn, :],
                                     zm_sb[:kn, kt, :])
                nc.tensor.matmul(d_ps[:], ones_bf[:kn, :], g_sb[:kn, :],
                                 start=(kt == 0), stop=(kt == 2))

            # ---- t1 = sum(D) ; t2 = sum(w2 * xs) ; out = t1 + t2 ----
            t1_sb = spool.tile([1, 1], F32, tag="t1")
            nc.vector.reduce_sum(t1_sb[:], d_ps[:], axis=mybir.AxisListType.X)
            junk2 = spool.tile([1, D], F32, tag="junk2")
            t2_sb = spool.tile([1, 1], F32, tag="t2")
            nc.vector.tensor_mul(junk2[:], rows_sb[0:1, 0:D],
                                 rows_sb[0:1, D:2 * D])
            nc.vector.reduce_sum(t2_sb[:], junk2[:], axis=mybir.AxisListType.X)
            out_sb = spool.tile([1, 1], F32, tag="out")
            nc.vector.tensor_add(out_sb[:], t1_sb[:], t2_sb[:])
            nc.sync.dma_start(out_d[:, :], out_sb[:])

    nc.compile()
    return nc


def _get_nc():
    global _CACHED_NC
    if _CACHED_NC is None:
        _CACHED_NC = _build_nc()
    return _CACHED_NC


def _fold(v, nt):
    """[nt*128] -> [128, nt] fold (v[t*128+p] -> out[p, t])."""
    return np.ascontiguousarray(v.reshape(nt, PT).T.astype(np.float32))


def kernel(**inputs):
    global LAST_RESULT
    x = np.asarray(inputs["x"], np.float32)
    Wq = np.asarray(inputs["Wq"], np.float32)
    bq = np.asarray(inputs["bq"], np.float32)
    Wk = np.asarray(inputs["Wk"], np.float32)
    bk = np.asarray(inputs["bk"], np.float32)
    Wv = np.asarray(inputs["Wv"], np.float32)
    bv = np.asarray(inputs["bv"], np.float32)
    Wc = np.asarray(inputs["Wc"], np.float32)
    bc = np.asarray(inputs["bc"], np.float32)

    w1, w2 = Wc[0, :D], Wc[0, D:]
    z = (x @ (Wv.T @ w1) + bv @ w1).astype(np.float32)
    rj = np.zeros(N, np.float32)
    rj[1:] = 1.0 / np.arange(1, N, dtype=np.float32)
    kidx = np.arange(N)[:, None]
    jidx = np.arange(N)[None, :]
    zmask = (z[:, None] * rj[None, :] * (kidx < jidx)).astype(np.float32)
    zmask_pad = np.zeros((3 * PT, N), np.float32)
    zmask_pad[:N] = zmask
    zmask_t = np.ascontiguousarray(
        zmask_pad.reshape(3, PT, N).transpose(1, 0, 2)).astype(BF16_NP)

    smalls = np.zeros((PT, 8), np.float32)
    smalls[:, 0:4] = _fold(bq, ND)
    smalls[:, 4:8] = _fold(bk, ND)

    xs_row = x.sum(axis=0, dtype=np.float64).astype(np.float32)

    common = {
        "xT": np.ascontiguousarray(x.T).astype(BF16_NP),
        "wqT": np.ascontiguousarray(Wq.T).astype(BF16_NP),
        "wkT": np.ascontiguousarray(Wk.T).astype(BF16_NP),
        "smalls": smalls,
        "zmask": zmask_t,
    }
    in_maps = []
    for c in range(NCORES):
        i0 = c * B
        iglob = (i0 + np.arange(B))[:, None]
        m = dict(common)
        m["xTb"] = np.ascontiguousarray(x[i0:i0 + B].T).astype(BF16_NP)
        m["bmask"] = (iglob < jidx).astype(np.float32).astype(BF16_NP)
        # t2 = w2 . sum_i x_i must be counted exactly once: only core 0
        rowv = np.zeros((1, 2 * D), np.float32)
        rowv[0, :D] = w2
        if c == 0:
            rowv[0, D:] = xs_row
        m["rows"] = rowv
        in_maps.append(m)

    nc = _get_nc()
    trace = bool(int(os.environ.get("KERNEL_TRACE", "0")))
    trace_cores = None
    if trace:
        try:
            _ensure_ntff_hook()
        except Exception as e:
            print(f"ntff hook shim failed ({e!r}); running untraced")
            trace = False
        if int(os.environ.get("KERNEL_TRACE_ALL", "0")):
            trace_cores = list(range(NCORES))
    res = run_bass_kernel_spmd(
        nc, in_maps, core_ids=list(range(NCORES)),
        trace=trace, trace_cores=trace_cores,
    )
    LAST_RESULT = res
    total = np.float64(0.0)
    for c in range(NCORES):
        total += np.float64(res.results[c]["out"][0, 0])
    total += np.float64(N) * np.float64(bc[0])
    return np.array([total], dtype=np.float32)
